# revision 1
# baseline (speedup 1.0000x reference)
"""Trainium2 Bass kernel for MockGCN segment-reduce problem.

Pipeline (per 8-way data-parallel shard, graphs grouped per shard):
  h1 = relu(x @ W_in + b_in)         [N, 64]
  h2 = relu(h1 @ W_h + b_h)          [N, 64]
  pooled[g] = mean_{i in g} h2[i]    [G, 64]
  out = pooled @ W_out + b_out       [G, 5]

Device layout: features-on-partitions ("T orientation"), nodes 2-packed
across the 128 partitions (64 feats x 2 node streams) and 4-interleaved
along the free axis so a column of the partial tensor holds 4 consecutive
nodes.  The host pads every segment to a multiple of 4 nodes, packs
x into xT_dev [128, C4], and the device emits per-4-node-block partial
sums P [128, C4] (bf16).  The host finishes the per-segment combine,
mean division, pad correction, and the tiny [G,64]@[64,5] matmul.

Stage map (per 512-col quantum = 2048 nodes):
  DMA in xT chunk [128, 512] bf16 (batched 4 quanta per dma_start)
  PE:  MM-A (rows 0-63, row-tiled)  -> psum ab[:, 0:512]    (h1raw stream0)
       MM-B (rows 64-127, row-tiled)-> psum ab[:, 512:1024] (h1raw stream1)
       (bf16 matmuls: fp32 runs at 1/4 rate on TRN2's PE)
  D1:  relu(ab + b1) -> rhs2 [128,1024] bf16 SBUF  (ScalarE, 1x fp32-psum read)
  PE:  MM x2 (K=128, blkdiag(W2,W2)) -> psum cd[:, 0:512], cd[:, 512:1024]
  D2:  relu(cd + b2) -> h2r [128,1024] bf16 SBUF   (VectorE; ScalarE steals
       every D2_ACT_EVERY-th quantum to balance the two drain engines)
  R:   pc = h2r[:,0:512] + h2r[:,512:1024]  (GpSimd TT; Pool is otherwise idle)
  DMA out pc -> partials[:, q*512:(q+1)*512] (batched 2 quanta per dma_start)

The PSUM->SBUF drains (D1/D2) are the hard bottleneck: 32M elements/core
through ScalarE (1x, 1.2GHz) + VectorE (1x for fp32-PSUM source, 0.96GHz).
"""

import sys

if "/opt/trn_rl_repo" not in sys.path:
    sys.path.insert(0, "/opt/trn_rl_repo")

from contextlib import ExitStack

import ml_dtypes
import numpy as np

N_CORES = 8
G_TOTAL = 8192
F_IN = 32
H_DIM = 64
Q_COLS = 512  # partial columns per quantum
PACK = 4  # nodes per partial column
DMA_BATCH = 2  # quanta per input DMA transfer
OUT_BATCH = 2  # quanta per output DMA transfer
D2_ACT_EVERY = 14  # ScalarE steals D2 from VectorE every k-th quantum
DEVICE_R = True  # sum the two node streams on device vs on host
DELAY_STEAL = False  # emit the ScalarE-stolen D2 one quantum late
R_ON_POOL = True  # device stream-sum engine: GpSimd (True) or VectorE (False)

_BUILD_CACHE: dict = {}
_LAST_IN_MAPS: list | None = None


def _build_program(c4: int):
    """Build + compile the 8-core SPMD Bass program for C4 partial columns."""
    import concourse.tile as tile
    from concourse import bacc, mybir

    f32 = mybir.dt.float32
    bf16 = mybir.dt.bfloat16
    Relu = mybir.ActivationFunctionType.Relu
    add_op = mybir.AluOpType.add
    max_op = mybir.AluOpType.max

    nq = c4 // Q_COLS
    assert c4 % Q_COLS == 0
    assert nq % DMA_BATCH == 0

    nc = bacc.Bacc(
        "TRN2",
        target_bir_lowering=False,
        debug=False,
        enable_asserts=False,
        num_devices=N_CORES,
    )

    xT = nc.dram_tensor("xT", [128, c4], bf16, kind="ExternalInput").ap()
    w1 = nc.dram_tensor("w1", [128, 128], bf16, kind="ExternalInput").ap()
    w2 = nc.dram_tensor("w2", [128, 128], bf16, kind="ExternalInput").ap()
    b1 = nc.dram_tensor("b1", [128, 1], f32, kind="ExternalInput").ap()
    b2 = nc.dram_tensor("b2", [128, 1], f32, kind="ExternalInput").ap()
    pw = c4 if DEVICE_R else 2 * c4
    pout = nc.dram_tensor("pout", [128, pw], bf16, kind="ExternalOutput").ap()

    with ExitStack() as ctx:
        tc = ctx.enter_context(tile.TileContext(nc))
        singles = ctx.enter_context(tc.tile_pool(name="singles", bufs=1))
        xpool = ctx.enter_context(tc.tile_pool(name="xc", bufs=3))
        rpool = ctx.enter_context(tc.tile_pool(name="rhs2", bufs=3))
        hpool = ctx.enter_context(tc.tile_pool(name="h2r", bufs=3))
        ppool = ctx.enter_context(tc.tile_pool(name="pc", bufs=3))
        abpool = ctx.enter_context(tc.tile_pool(name="ab", bufs=2, space="PSUM"))
        cdpool = ctx.enter_context(tc.tile_pool(name="cd", bufs=2, space="PSUM"))

        w1sb = singles.tile([128, 128], bf16)
        w2sb = singles.tile([128, 128], bf16)
        b1sb = singles.tile([128, 1], f32)
        b2sb = singles.tile([128, 1], f32)
        # Weights ride the SWDGE (gpsimd) ring so the first x chunk leads the
        # HWDGE (sync) FIFO instead of queueing behind four small transfers.
        nc.gpsimd.dma_start(out=w1sb, in_=w1)
        nc.gpsimd.dma_start(out=w2sb, in_=w2)
        nc.gpsimd.dma_start(out=b1sb, in_=b1)
        nc.gpsimd.dma_start(out=b2sb, in_=b2)

        # Pre-warm the ScalarE activation table (~2.7us PSEUDO_LOAD_ACT_FUNC_SET
        # attaches to the first ACTIVATE) so it overlaps the first x-chunk DMA.
        warm = singles.tile([128, 1], f32)
        nc.vector.memset(warm, 0.0)
        nc.scalar.activation(warm, warm, Relu)

        Q = Q_COLS
        xc = None
        h2r = None
        pc = None
        pending = None  # delayed ScalarE-stolen D2: (cd, dst, pc, h2r, q)
        for q in range(nq):
            if q % DMA_BATCH == 0:
                xc = xpool.tile([128, DMA_BATCH * Q], bf16)
                nc.sync.dma_start(
                    out=xc, in_=xT[:, q * Q : (q + DMA_BATCH) * Q]
                )
            jx = (q % DMA_BATCH) * Q
            if q % OUT_BATCH == 0:
                h2r = hpool.tile([128, OUT_BATCH * 2 * Q], bf16)
            jh = (q % OUT_BATCH) * 2 * Q

            ab = abpool.tile([128, 2 * Q], f32)
            # Stage 1: two row-tiled matmuls run concurrently on PE.
            nc.tensor.matmul(
                out=ab[:, 0:Q],
                lhsT=w1sb[0:64, :],
                rhs=xc[0:64, jx : jx + Q],
                start=True,
                stop=True,
            )
            nc.tensor.matmul(
                out=ab[:, Q : 2 * Q],
                lhsT=w1sb[64:128, :],
                rhs=xc[64:128, jx : jx + Q],
                start=True,
                stop=True,
            )

            rhs2 = rpool.tile([128, 2 * Q], bf16)
            nc.scalar.activation(rhs2, ab, Relu, bias=b1sb)

            def finish_quantum(qq, dst_t, pc_t, h2r_t, fold_on_dve=False):
                if DEVICE_R:
                    r_eng = (
                        nc.vector
                        if (fold_on_dve or not R_ON_POOL)
                        else nc.gpsimd
                    )
                    r_eng.tensor_add(
                        pc_t[:, (qq % OUT_BATCH) * Q : (qq % OUT_BATCH + 1) * Q],
                        dst_t[:, 0:Q],
                        dst_t[:, Q : 2 * Q],
                    )
                if qq % OUT_BATCH == OUT_BATCH - 1:
                    q0 = qq - (OUT_BATCH - 1)
                    if DEVICE_R:
                        nc.sync.dma_start(
                            out=pout[:, q0 * Q : (qq + 1) * Q], in_=pc_t
                        )
                    else:
                        nc.sync.dma_start(
                            out=pout[:, 2 * q0 * Q : 2 * (qq + 1) * Q], in_=h2r_t
                        )

            # Flush a D2 stolen by ScalarE one quantum ago (its deps are long
            # ready, so ScalarE never stalls on this quantum's stage-2).
            if pending is not None:
                cd_p, dst_p, pc_p, h2r_p, q_p = pending
                nc.scalar.activation(dst_p, cd_p, Relu, bias=b2sb)
                finish_quantum(q_p, dst_p, pc_p, h2r_p)
                pending = None

            cd = cdpool.tile([128, 2 * Q], f32)
            nc.tensor.matmul(
                out=cd[:, 0:Q], lhsT=w2sb, rhs=rhs2[:, 0:Q], start=True, stop=True
            )
            nc.tensor.matmul(
                out=cd[:, Q : 2 * Q],
                lhsT=w2sb,
                rhs=rhs2[:, Q : 2 * Q],
                start=True,
                stop=True,
            )

            if q % OUT_BATCH == 0:
                pc = ppool.tile([128, OUT_BATCH * Q], bf16)
            dst = h2r[:, jh : jh + 2 * Q]
            if (
                q % D2_ACT_EVERY == D2_ACT_EVERY - 1
                and q % OUT_BATCH == OUT_BATCH - 1
                and q < nq - 1
            ):
                if DELAY_STEAL:
                    pending = (cd, dst, pc, h2r, q)
                else:
                    nc.scalar.activation(dst, cd, Relu, bias=b2sb)
                    finish_quantum(q, dst, pc, h2r)
            else:
                nc.vector.tensor_scalar(dst, cd, b2sb, 0.0, add_op, max_op)
                finish_quantum(q, dst, pc, h2r)

    nc.compile()
    return nc


def _get_program(c4: int):
    if c4 not in _BUILD_CACHE:
        _BUILD_CACHE[c4] = _build_program(c4)
    return _BUILD_CACHE[c4]


def kernel(x, batch, num_graphs, W_in, b_in, W_h, b_h, W_out, b_out):
    from concourse import bass_utils

    x = np.asarray(x, dtype=np.float32)
    batch = np.asarray(batch).astype(np.int64)
    g_total = int(num_graphs)
    W_in = np.asarray(W_in, dtype=np.float32)
    b_in = np.asarray(b_in, dtype=np.float32)
    W_h = np.asarray(W_h, dtype=np.float32)
    b_h = np.asarray(b_h, dtype=np.float32)
    W_out = np.asarray(W_out, dtype=np.float32)
    b_out = np.asarray(b_out, dtype=np.float32)

    if batch.size and np.any(np.diff(batch) < 0):
        order = np.argsort(batch, kind="stable")
        x = x[order]
        batch = batch[order]

    n_nodes, f_in = x.shape
    h_dim = W_in.shape[1]
    assert f_in == F_IN and h_dim == H_DIM
    assert g_total % N_CORES == 0
    g_per_core = g_total // N_CORES

    counts = np.bincount(batch, minlength=g_total).astype(np.int64)
    node_starts = np.concatenate([[0], np.cumsum(counts)])  # [G+1]

    # Per-graph padded counts (multiple of PACK).
    pc_counts = (counts + PACK - 1) // PACK * PACK

    # Per-core geometry.
    core_g0 = [c * g_per_core for c in range(N_CORES)]
    core_pad_tot = [
        int(pc_counts[c * g_per_core : (c + 1) * g_per_core].sum())
        for c in range(N_CORES)
    ]
    c4_per_core = [t // PACK for t in core_pad_tot]
    align = Q_COLS * DMA_BATCH
    c4 = max(c4_per_core)
    c4 = (c4 + align - 1) // align * align  # uniform, DMA-batch aligned

    # Constant tensors shared by all cores.
    w1blk = np.zeros((128, 128), dtype=np.float32)
    w1blk[0:32, 0:64] = W_in
    w1blk[32:64, 64:128] = W_in
    w1blk[64:96, 0:64] = W_in
    w1blk[96:128, 64:128] = W_in
    w1blk = w1blk.astype(ml_dtypes.bfloat16)
    w2blk = np.zeros((128, 128), dtype=np.float32)
    w2blk[0:64, 0:64] = W_h
    w2blk[64:128, 64:128] = W_h
    w2blk = w2blk.astype(ml_dtypes.bfloat16)
    b1cat = np.tile(b_in, 2).reshape(128, 1).astype(np.float32)
    b2cat = np.tile(b_h, 2).reshape(128, 1).astype(np.float32)

    # Per-core packed inputs.
    in_maps = []
    for c in range(N_CORES):
        g0 = core_g0[c]
        g1 = g0 + g_per_core
        s, e = int(node_starts[g0]), int(node_starts[g1])
        cnt_c = counts[g0:g1]
        pc_c = pc_counts[g0:g1]
        pad_starts = np.concatenate([[0], np.cumsum(pc_c)])  # [g_per_core+1]

        x_padded = np.zeros((c4 * PACK, f_in), dtype=np.float32)
        if e > s:
            local_batch = batch[s:e] - g0
            # dst = pad_start of graph + index within graph
            dst = pad_starts[local_batch] + (
                np.arange(s, e) - node_starts[g0 + local_batch]
            )
            x_padded[dst] = x[s:e]
        xT_dev = (
            x_padded.reshape(c4, PACK, f_in).transpose(1, 2, 0).reshape(128, c4)
        )
        xT_dev = np.ascontiguousarray(xT_dev).astype(ml_dtypes.bfloat16)
        in_maps.append(
            {
                "xT": xT_dev,
                "w1": w1blk,
                "w2": w2blk,
                "b1": b1cat,
                "b2": b2cat,
            }
        )

    global _LAST_IN_MAPS
    _LAST_IN_MAPS = in_maps

    nc = _get_program(c4)
    res = bass_utils.run_bass_kernel_spmd(
        nc, in_maps, core_ids=list(range(N_CORES))
    )

    # Pad-node contribution, exactly as the device computes it for x=0 rows:
    # h1raw = 0 (fp32 psum) -> D1: bf16(relu(b1)) -> S2 bf16 matmul (fp32 acc)
    # -> D2: bf16(relu(. + b2)).
    bf = ml_dtypes.bfloat16
    h1p = np.maximum(b_in, 0.0).astype(bf).astype(np.float32)
    w2bf = W_h.astype(bf).astype(np.float32)
    vpad = np.maximum(h1p @ w2bf + b_h, 0.0).astype(bf).astype(np.float32)
    vpad_bf = vpad

    out = np.zeros((g_total, W_out.shape[1]), dtype=np.float32)
    for c in range(N_CORES):
        g0 = core_g0[c]
        g1 = g0 + g_per_core
        cnt_c = counts[g0:g1].astype(np.float64)
        pc_c = pc_counts[g0:g1]
        pad_starts = np.concatenate([[0], np.cumsum(pc_c)])
        col_starts = pad_starts // PACK  # [g_per_core+1]

        nq = c4 // Q_COLS
        P = np.asarray(res.results[c]["pout"]).astype(np.float32)
        if DEVICE_R:
            R1 = P[0:64, :] + P[64:128, :]  # [64, c4]
        else:
            Pr = P.reshape(128, nq, 2, Q_COLS)
            # fold partition halves (node pairs) and streams
            R1 = (Pr[0:64] + Pr[64:128]).sum(axis=2).reshape(64, c4)
        cs = np.concatenate(
            [np.zeros((64, 1)), np.cumsum(R1.astype(np.float64), axis=1)], axis=1
        )  # [64, c4+1]
        seg_sum = (cs[:, col_starts[1:]] - cs[:, col_starts[:-1]]).T  # [g, 64]

        n_pad = (pc_c - counts[g0:g1]).astype(np.float64)
        seg_sum = seg_sum - n_pad[:, None] * vpad_bf[None, :].astype(np.float64)
        denom = np.maximum(cnt_c, 1.0)
        mean = seg_sum / denom[:, None]
        mean[cnt_c == 0] = 0.0
        out[g0:g1] = mean.astype(np.float32) @ W_out + b_out

    return out

